# revision 2
# baseline (speedup 1.0000x reference)
"""Trainium2 Bass kernel for CRKT layer (decay-reweighted causal attention), v2.

Math per batch b (one NeuronCore per batch element, 8 cores):
  q = query @ Wq.T + bq ; k = key_in @ Wq.T + bq ; v = value @ Wv.T + bv
  s = q k^T  (per head, causal; 1/sqrt(dk) folded into exp scales)
  expS = exp(0.125 s); Z = rowsum; suffix = Z - cumsum(expS)
  te = exp((lam/Z) * (cumsum - Z) * (i-j))     [= exp(-lam*dist)]
  P2 = exp(0.125 * s * te); alpha = P2 / rowsum(P2)
  out = alpha @ v ; y = LN(out @ Wo.T + bo) * gamma + beta

v2 structure (vs v1 baseline):
  - QK^T computed ONCE per (h,t); s kept in PSUM and reused for s2 = s*te
    (DVE in-place multiply into PSUM).
  - alpha^T strips produced by ONE strided dma_start_transpose per (h,t)
    (blocked 128x128 transpose with 3D dst AP) into a packed per-head
    strips buffer -- replaces 288 per-block DMA transposes (HWDGE relief).
  - P2 normalized in [i,j] layout with 4x-mode bf16 tensor_scalar_mul.
  - stt pass (cumsum-Z)*d split DVE/Pool for engine balance; d tiles in f16.
  - Batched DMA loads (one instruction per input/weight tensor), batched
    final store.
"""

import sys

for _p in ("/opt/trn_rl_repo",):
    if _p not in sys.path:
        sys.path.insert(0, _p)

import numpy as np

import concourse.bass as bass
import concourse.mybir as mybir
import concourse.tile as tile
from concourse import bacc, bass_utils
from concourse.masks import make_identity

F32 = mybir.dt.float32
F32R = mybir.dt.float32r
BF16 = mybir.dt.bfloat16
F16 = mybir.dt.float16
AL = mybir.AluOpType
AF = mybir.ActivationFunctionType

S, DIM, H, DK = 1024, 512, 8, 64
T = S // 128        # 8 i-tiles
NB = S // 128       # 8 j-blocks
NEGBIG = -1e30

_CACHE = {}


def _chunks(total, step):
    return [(a, min(a + step, total)) for a in range(0, total, step)]


def build():
    nc = bacc.Bacc("TRN2", target_bir_lowering=False, debug=False, num_devices=8)

    d_query = nc.dram_tensor("query", [S, DIM], F32, kind="ExternalInput")
    d_key = nc.dram_tensor("key_in", [S, DIM], F32, kind="ExternalInput")
    d_value = nc.dram_tensor("value", [S, DIM], F32, kind="ExternalInput")
    d_wq = nc.dram_tensor("Wq", [DIM, DIM], F32, kind="ExternalInput")
    d_wv = nc.dram_tensor("Wv", [DIM, DIM], F32, kind="ExternalInput")
    d_wo = nc.dram_tensor("Wo", [DIM, DIM], F32, kind="ExternalInput")
    d_bq = nc.dram_tensor("bq", [1, DIM], F32, kind="ExternalInput")
    d_bv = nc.dram_tensor("bv", [1, DIM], F32, kind="ExternalInput")
    d_bo = nc.dram_tensor("bo", [1, DIM], F32, kind="ExternalInput")
    d_dec = nc.dram_tensor("decay", [1, H], F32, kind="ExternalInput")
    d_gam = nc.dram_tensor("gamma", [1, DIM], F32, kind="ExternalInput")
    d_bet = nc.dram_tensor("beta", [1, DIM], F32, kind="ExternalInput")
    d_out = nc.dram_tensor("out", [S, DIM], F32, kind="ExternalOutput")

    with tile.TileContext(nc) as tc:
        _body(nc, tc, d_query, d_key, d_value, d_wq, d_wv, d_wo,
              d_bq, d_bv, d_bo, d_dec, d_gam, d_bet, d_out)

    nc.compile()
    return nc


def _body(nc, tc, d_query, d_key, d_value, d_wq, d_wv, d_wo,
          d_bq, d_bv, d_bo, d_dec, d_gam, d_bet, d_out):
    import contextlib
    ctx = contextlib.ExitStack()
    with ctx:
        const = ctx.enter_context(tc.tile_pool(name="const", bufs=1))
        persist = ctx.enter_context(tc.tile_pool(name="persist", bufs=1))

        # ---- constants ----
        ident = const.tile([128, 128], F32)
        make_identity(nc, ident[:])
        identr = const.tile([128, 128], F32R)
        nc.vector.tensor_copy(identr[:], ident[:])
        cmask = const.tile([128, 128], F32)
        nc.gpsimd.memset(cmask[:], 0.0)
        nc.gpsimd.affine_select(
            out=cmask[:], in_=cmask[:], compare_op=AL.is_ge, fill=NEGBIG,
            base=0, channel_multiplier=1, pattern=[[-1, 128]])
        cmaskr = const.tile([128, 128], F32R)
        nc.vector.tensor_copy(cmaskr[:], cmask[:])
        eps_t = const.tile([128, 1], F32)
        nc.vector.memset(eps_t[:], 1e-5)

        bv_pkr = const.tile([128, 4], BF16)
        bo_rowr = const.tile([1, DIM], F32R)
        bq_sb = const.tile([128, 4], F32)   # col m = bq[128m:128(m+1)]
        nc.sync.dma_start(bq_sb[:], d_bq[0, :].rearrange("(m p) -> p m", p=128))
        bv_pk = const.tile([128, 4], F32)   # col r = bv[128r:128(r+1)]
        nc.sync.dma_start(bv_pk[:], d_bv[0, :].rearrange("(m p) -> p m", p=128))
        nc.vector.tensor_copy(bv_pkr[:], bv_pk[:])
        bo_row = const.tile([1, DIM], F32)
        nc.sync.dma_start(bo_row[:], d_bo[0:1, :])
        nc.vector.tensor_copy(bo_rowr[:], bo_row[:])
        ones1f = const.tile([1, 128], F32)
        nc.vector.memset(ones1f[:], 1.0)
        ones1 = const.tile([1, 128], F32R)
        nc.vector.tensor_copy(ones1[:], ones1f[:])
        onecol = const.tile([1, 1], F32R)
        nc.vector.tensor_copy(onecol[:], ones1f[:, 0:1])
        gam_bc = const.tile([128, DIM], F32)
        nc.sync.dma_start(gam_bc[:], d_gam.ap().to_broadcast((128, DIM)))
        bet_bc = const.tile([128, DIM], F32)
        nc.sync.dma_start(bet_bc[:], d_bet.ap().to_broadcast((128, DIM)))

        lam = const.tile([128, H], F32)     # |decay_h| broadcast down partitions
        nc.sync.dma_start(lam[:], d_dec.ap().to_broadcast((128, H)))
        nc.scalar.activation(lam[:], lam[:], AF.Abs)

        # D tiles: d_t[p, j] = 128 t + p - j  (= i - j), j in [0, L_t); f16 exact
        d_tiles = []
        for t in range(T):
            L = 128 * (t + 1)
            dt_ = const.tile([128, L], F16, tag=f"dti_{t}", name=f"dti_{t}")
            nc.gpsimd.iota(dt_[:], pattern=[[-1, L]], base=128 * t,
                           channel_multiplier=1,
                           allow_small_or_imprecise_dtypes=True)
            d_tiles.append(dt_)

        # ---- persistent tensors ----
        qt = [persist.tile([128, S], F32R, tag=f"qt{g}", name=f"qt{g}") for g in range(4)]
        kt = [persist.tile([128, S], F32R, tag=f"kt{g}", name=f"kt{g}") for g in range(4)]
        v_sb = [persist.tile([128, DIM], BF16, tag=f"v{t}", name=f"v{t}") for t in range(T)]
        ot_sb = [persist.tile([128, S], BF16, tag=f"ot{g}", name=f"ot{g}") for g in range(4)]
        wot = [persist.tile([128, DIM], BF16, tag=f"wot{g}", name=f"wot{g}") for g in range(4)]

        # ---- load inputs/weights (batched), transpose, project ----
        with tc.tile_pool(name="stage", bufs=1) as stage, \
             tc.tile_pool(name="pstage", bufs=2, space="PSUM") as pstage:

            # batched loads: xall[p, t*DIM + d] = x[128t + p, d]
            # staging tiles shared across inputs (sequential reuse)
            def load_x(dram, nm):
                xa = stage.tile([128, T * DIM], F32, tag="xa", name=nm, bufs=2)
                nc.sync.dma_start(
                    xa[:].rearrange("p (t d) -> p t d", d=DIM),
                    dram.ap().rearrange("(t p) d -> p t d", p=128))
                return xa

            def load_w(dram, nm):
                wa = stage.tile([128, 4 * DIM], F32, tag="wa", name=nm, bufs=2)
                nc.sync.dma_start(
                    wa[:].rearrange("p (r d) -> p r d", d=DIM),
                    dram.ap().rearrange("(r p) d -> p r d", p=128))
                return wa

            def transp_w(wa, name, wot_dst=None):
                cols = []
                for c in range(4):
                    pt = pstage.tile([128, DIM], F32, tag="ppj")
                    for r in range(4):
                        nc.tensor.transpose(
                            pt[:, 128 * r:128 * (r + 1)],
                            wa[:, DIM * r + 128 * c:DIM * r + 128 * (c + 1)],
                            ident[:])
                    if wot_dst is not None:
                        dst = wot_dst[c]
                        nc.scalar.activation(dst[:], pt[:], AF.Identity)
                    else:
                        dst = persist.tile([128, DIM], F32R, tag=f"{name}t{c}",
                                           name=f"{name}t{c}")
                        nc.vector.tensor_copy(dst[:], pt[:])
                    cols.append(dst)
                return cols

            def transp_x(xa, nm, on_act):
                xt = []
                for dblk in range(4):
                    ptx = pstage.tile([128, S], F32, tag="ptx")
                    for t in range(T):
                        nc.tensor.transpose(
                            ptx[:, 128 * t:128 * (t + 1)],
                            xa[:, DIM * t + 128 * dblk:DIM * t + 128 * (dblk + 1)],
                            ident[:])
                    xtd = stage.tile([128, S], F32R, tag=f"xt{dblk}",
                                     name=f"{nm}{dblk}")
                    if on_act:
                        nc.scalar.activation(xtd[:], ptx[:], AF.Identity)
                    else:
                        nc.vector.tensor_copy(xtd[:], ptx[:])
                    xt.append(xtd)
                return xt

            def proj_qk(xt, dst_tiles, gs):
                for g in gs:
                    for half in range(2):
                        pp = pstage.tile([128, DIM], F32, tag="ppj")
                        for kk in range(4):
                            nc.tensor.matmul(
                                pp[:],
                                wqt[kk][:, 128 * g:128 * (g + 1)],
                                xt[kk][:, DIM * half:DIM * (half + 1)],
                                start=(kk == 0), stop=(kk == 3))
                        dst = dst_tiles[g][:, DIM * half:DIM * (half + 1)]
                        nc.scalar.activation(dst, pp[:], AF.Identity,
                                             bias=bq_sb[:, g:g + 1])

            wa_q = load_w(d_wq, "wa_q")
            xa_q = load_x(d_query, "xa_q")
            wqt = transp_w(wa_q, "wq")
            xt_q = transp_x(xa_q, "xtq", on_act=False)
            xa_k = load_x(d_key, "xa_k")
            xt_k = transp_x(xa_k, "xtk", on_act=True)
            proj_qk(xt_q, qt, [0, 1, 2, 3])
            proj_qk(xt_k, kt, [0, 1, 2, 3])
            wa_v = load_w(d_wv, "wa_v")
            wvt = transp_w(wa_v, "wv")
            xa_v = persist.tile([128, T * DIM], F32, tag="xav", name="xa_v")
            nc.sync.dma_start(
                xa_v[:].rearrange("p (t d) -> p t d", d=DIM),
                d_value.ap().rearrange("(t p) d -> p t d", p=128))
            wa_o = persist.tile([128, 4 * DIM], F32, tag="wao", name="wa_o")
            nc.sync.dma_start(
                wa_o[:].rearrange("p (r d) -> p r d", d=DIM),
                d_wo.ap().rearrange("(r p) d -> p r d", p=128))
            xt_v = [persist.tile([128, S], F32R, tag=f"xtv{dblk}",
                                 name=f"xtv{dblk}") for dblk in range(4)]

            b2row = persist.tile([1, DIM], F32R, tag="b2row", name="b2row")
            prologue_tail = []
            late = {}

            def _xtv(dblk):
                def run():
                    ptx = late["pss"].tile([128, S], F32, tag="sb",
                                           name="ptxv")
                    for t in range(T):
                        nc.tensor.transpose(
                            ptx[:, 128 * t:128 * (t + 1)],
                            xa_v[:, DIM * t + 128 * dblk:DIM * t + 128 * (dblk + 1)],
                            ident[:])
                    nc.vector.tensor_copy(xt_v[dblk][:], ptx[:])
                return run

            for dblk in range(4):
                prologue_tail.append(_xtv(dblk))

            def _vproj(t):
                def run():
                    pp = late["pss"].tile([128, DIM], F32, tag="sb",
                                          name="ppv")
                    for kk in range(4):
                        nc.tensor.matmul(
                            pp[:], xt_v[kk][:, 128 * t:128 * (t + 1)],
                            wvt[kk][:],
                            start=(kk == 0), stop=(kk == 3))
                    nc.scalar.activation(v_sb[t][:], pp[:], AF.Identity)
                return run
            for t in range(T):
                prologue_tail.append(_vproj(t))

            def _wo(c):
                def run():
                    pt = late["pss"].tile([128, DIM], F32, tag="sb",
                                          name="ptwo")
                    for r in range(4):
                        nc.tensor.transpose(
                            pt[:, 128 * r:128 * (r + 1)],
                            wa_o[:, DIM * r + 128 * c:DIM * r + 128 * (c + 1)],
                            ident[:])
                    nc.scalar.activation(wot[c][:], pt[:], AF.Identity)
                return run
            for c in range(4):
                prologue_tail.append(_wo(c))

            def _b2():
                # b2 = Wo bv + bo (bv folds through PV: alpha rows sum to 1)
                b2ps = late["pss"].tile([1, DIM], F32, tag="sb", name="b2ps")
                for g in range(4):
                    nc.tensor.matmul(b2ps[:], bv_pkr[:, g:g + 1],
                                     wot[g][:], start=(g == 0), stop=False)
                nc.tensor.matmul(b2ps[:], onecol[:], bo_rowr[:],
                                 start=False, stop=True)
                nc.vector.tensor_copy(b2row[:], b2ps[:])
            prologue_tail.append(_b2)

        # ---- attention ----
        with tc.tile_pool(name="pssA", bufs=1, space="PSUM") as pssA, \
             tc.tile_pool(name="pssB", bufs=2, space="PSUM") as pssB, \
             tc.tile_pool(name="psot", bufs=1, space="PSUM") as psot, \
             tc.tile_pool(name="sbA", bufs=4) as sbA, \
             tc.tile_pool(name="sbS", bufs=8) as sbS, \
             tc.tile_pool(name="sbP", bufs=2) as sbP, \
             tc.tile_pool(name="sbStr", bufs=1) as sbStr:

            late["pss"] = pssB
            # strided views over packed strips: strips_h[:, b, i] (b-major)
            strips = {}

            def qk(h, t, ps):
                g, off = h // 2, 64 * (h % 2)
                L = 128 * (t + 1)
                for (j0, j1) in _chunks(L, 512):
                    nc.tensor.matmul(
                        ps[:, j0:j1],
                        qt[g][off:off + 64, 128 * t:128 * (t + 1)],
                        kt[g][off:off + 64, j0:j1],
                        start=True, stop=(j1 < L))
                nc.tensor.matmul(ps[:, 128 * t:L], identr[:], cmaskr[:],
                                 start=False, stop=True)

            st = {}

            def stage_exp1(h, t, par):
                L = 128 * (t + 1)
                ps = pssA.tile([128, S], F32, tag="sa", name=f"psa_{h}_{t}")
                qk(h, t, ps)
                expS = sbA.tile([128, S], F32, tag=f"expS{par}",
                                name=f"e_{h}_{t}", bufs=2)
                nc.scalar.activation(expS[:, :L], ps[:, :L], AF.Exp,
                                     scale=0.125)
                st[(h, t)] = {"expS": expS}

            def stage_scan(h, t, par):
                L = 128 * (t + 1)
                d = st[(h, t)]
                scan = d["expS"]
                nc.vector.tensor_tensor_scan(
                    scan[:, :L], d["expS"][:, :L], d["expS"][:, :L], 0.0,
                    op0=AL.add, op1=AL.bypass)
                sc1 = sbS.tile([128, 1], F32, tag="sc1", name=f"s1_{h}_{t}")
                nc.vector.reciprocal(sc1[:], scan[:, L - 1:L])
                nc.vector.tensor_scalar_mul(sc1[:], in0=sc1[:],
                                            scalar1=lam[:, h:h + 1])
                d["sc1"] = sc1
                # stt: u = (scan - Z) * d ; small tiles on Pool (2-op)
                z1 = scan[:, L - 1:L]
                if t < 4:
                    nc.gpsimd.tensor_scalar_sub(scan[:, :L], in0=scan[:, :L],
                                                scalar1=z1)
                    nc.gpsimd.tensor_mul(scan[:, :L], scan[:, :L],
                                         d_tiles[t][:, :L])
                else:
                    nc.vector.scalar_tensor_tensor(
                        scan[:, :L], in0=scan[:, :L], scalar=z1,
                        in1=d_tiles[t][:, :L], op0=AL.subtract, op1=AL.mult)

            def stage_te(h, t, par):
                L = 128 * (t + 1)
                d = st[(h, t)]
                te = sbP.tile([128, S], BF16, tag=f"te{par}",
                              name=f"te_{h}_{t}", bufs=2)
                nc.scalar.activation(te[:, :L], d["expS"][:, :L], AF.Exp,
                                     scale=d["sc1"][:])
                d["te"] = te

            def stage_s2(h, t, par):
                L = 128 * (t + 1)
                d = st[(h, t)]
                ps = pssB.tile([128, S], F32, tag="sb", name=f"psb_{h}_{t}")
                qk(h, t, ps)
                nc.vector.tensor_mul(ps[:, :L], ps[:, :L], d["te"][:, :L])
                d["ps"] = ps

            def stage_exp2(h, t, par):
                L = 128 * (t + 1)
                d = st[(h, t)]
                p2 = sbA.tile([128, S], BF16, tag=f"p2{par}",
                              name=f"p2_{h}_{t}", bufs=2)
                z2 = sbS.tile([128, 1], F32, tag="z2", name=f"z2_{h}_{t}")
                nc.scalar.activation(p2[:, :L], d["ps"][:, :L], AF.Exp,
                                     scale=0.125, accum_out=z2[:])
                rz2 = sbS.tile([128, 1], F32, tag="rz2", name=f"r2_{h}_{t}")
                nc.vector.reciprocal(rz2[:], z2[:])
                nc.vector.tensor_scalar_mul(p2[:, :L], in0=p2[:, :L],
                                            scalar1=rz2[:])
                dst = strips[h][:].rearrange("p (b c) -> p b c", c=S)
                dst = dst[:, :t + 1, 128 * t:128 * (t + 1)]
                nc.sync.dma_start_transpose(dst, p2[:, :L])
                del st[(h, t)]

            pair_pot = {}

            def ot_chunk(h, c):
                g, off = h // 2, 64 * (h % 2)
                i0 = 512 * c
                if h % 2 == 0:
                    pair_pot[c] = psot.tile([128, DIM], F32, tag="ot",
                                            name=f"pot{c}_{h}")
                pot = pair_pot[c]
                bs = [b for b in range(NB) if 128 * b < i0 + 512]
                sall = strips[h]
                for b in bs:
                    a0 = max(0, 128 * b - i0)
                    nc.tensor.matmul(
                        pot[off:off + 64, a0:512],
                        v_sb[b][:, 64 * h:64 * h + 64],
                        sall[:, S * b + i0 + a0:S * b + i0 + 512],
                        start=(b == 0), stop=(b == bs[-1]),
                        tile_position=(0, off))
                if h % 2 == 1:
                    gg = h // 2
                    nc.scalar.activation(ot_sb[gg][:, i0:i0 + 512], pot[:],
                                         AF.Identity)

            for hp in range(4):
                h0, h1 = 2 * hp, 2 * hp + 1
                strips[h0] = sbStr.tile([128, NB * S], BF16, tag="str0",
                                        name=f"str_{h0}")
                strips[h1] = sbStr.tile([128, NB * S], BF16, tag="str1",
                                        name=f"str_{h1}")
                jobs = []
                for t in range(T):
                    jobs.append((h0, t, 0))
                    jobs.append((h1, t, 1))

                NJ = len(jobs)
                for n in range(NJ + 4):
                    if n < NJ:
                        stage_exp1(*jobs[n])
                    if 1 <= n < NJ + 1:
                        stage_scan(*jobs[n - 1])
                    if 2 <= n < NJ + 2:
                        stage_te(*jobs[n - 2])
                    if 3 <= n < NJ + 3:
                        stage_s2(*jobs[n - 3])
                    if 4 <= n:
                        stage_exp2(*jobs[n - 4])
                    if prologue_tail and n >= 1:
                        prologue_tail.pop(0)()
                        if n >= 5 and prologue_tail:
                            prologue_tail.pop(0)()
                    if n == 11:
                        ot_chunk(h0, 0)
                        ot_chunk(h1, 0)
                ot_chunk(h0, 1)
                ot_chunk(h1, 1)

            # ---- output projection + LayerNorm ----
            for t in range(T):
                psy = pssB.tile([128, DIM], F32, tag="sb", name="psy")
                for g in range(4):
                    nc.tensor.matmul(psy[:],
                                     ot_sb[g][:, 128 * t:128 * (t + 1)],
                                     wot[g][:], start=(g == 0), stop=False)
                nc.tensor.matmul(psy[:], ones1[:], b2row[:],
                                 start=False, stop=True)
                stats = sbS.tile([128, 6], F32, tag="bst")
                nc.vector.bn_stats(out=stats[:], in_=psy[:])
                mv = sbS.tile([128, 2], F32, tag="bmv")
                nc.vector.bn_aggr(out=mv[:], in_=stats[:])
                rstd = sbS.tile([128, 1], F32, tag="rstd")
                nc.scalar.activation(rstd[:], mv[:, 1:2], AF.Sqrt,
                                     bias=eps_t[:])
                nc.vector.reciprocal(rstd[:], rstd[:])
                y2t = sbA.tile([128, DIM], F32, tag="y2", bufs=2,
                               name=f"y2_{t}")
                nc.vector.tensor_scalar(out=y2t[:], in0=psy[:],
                                        scalar1=mv[:, 0:1], scalar2=rstd[:],
                                        op0=AL.subtract, op1=AL.mult)
                nc.gpsimd.tensor_mul(y2t[:], y2t[:], gam_bc[:])
                nc.gpsimd.tensor_add(y2t[:], y2t[:], bet_bc[:])
                nc.sync.dma_start(d_out[128 * t:128 * (t + 1), :], y2t[:])


def kernel(**inputs):
    query = np.asarray(inputs["query"], np.float32)
    key_in = np.asarray(inputs["key_in"], np.float32)
    value = np.asarray(inputs["value"], np.float32)
    B = query.shape[0]
    assert query.shape == (B, S, DIM)

    if "nc" not in _CACHE:
        _CACHE["nc"] = build()
    nc = _CACHE["nc"]

    base = {
        "Wq": np.asarray(inputs["Wq"], np.float32),
        "Wv": np.asarray(inputs["Wv"], np.float32),
        "Wo": np.asarray(inputs["Wo"], np.float32),
        "bq": np.asarray(inputs["bq"], np.float32).reshape(1, DIM),
        "bv": np.asarray(inputs["bv"], np.float32).reshape(1, DIM),
        "bo": np.asarray(inputs["bo"], np.float32).reshape(1, DIM),
        "decay": np.asarray(inputs["decay"], np.float32).reshape(1, H),
        "gamma": np.asarray(inputs["gamma"], np.float32).reshape(1, DIM),
        "beta": np.asarray(inputs["beta"], np.float32).reshape(1, DIM),
    }
    in_maps = []
    for c in range(8):
        b = min(c, B - 1)
        m = dict(base)
        m["query"] = np.ascontiguousarray(query[b])
        m["key_in"] = np.ascontiguousarray(key_in[b])
        m["value"] = np.ascontiguousarray(value[b])
        in_maps.append(m)

    res = bass_utils.run_bass_kernel_spmd(nc, in_maps, core_ids=list(range(8)))
    out = np.stack([res.results[c]["out"] for c in range(B)], 0)
    return out.astype(np.float32)
